# revision 14
# baseline (speedup 1.0000x reference)
"""Causal single-head attention on 8 Trainium2 NeuronCores.

Problem: x[4, 4096, 1024], Wq/Wk/Wv[1024, 64] ->
  out = softmax(causal(Q K^T / 8)) V   per batch, fp32.

Sharding: core i handles batch b = i//2 with query-chunk parity p = i%2
(512-wide query chunks; core p owns global chunks {p, 2+p, 4+p, 6+p}).
SPMD: the program is identical on all cores; parity enters only through
data (which x chunks appear in xqT, the additive mask buffer, and host
assembly).

v2 design (from trace analysis of the fp32r baseline: fp32r streams 2
cycles/column on the PE, LDWEIGHTS churn dominated the V projection,
and ACT bubbles were paid per 512-col exp tile):
  - all matmul operands in bf16 (1 cy/col expected): halves x DMA too
  - K and V projected together as one stacked [Wk|Wv] matmul per chunk
    (V^T rows 64:128); V^T -> V natural via DMA-engine XBAR transpose
  - Q projected only for the 4 owned chunks from a separate xqT upload
    (uniform program; +4MiB DMA instead of +16.4k PE columns)
  - attention processes key tiles in PAIRS: one [128,1024] exp per pair
    (halves ACT instruction bubbles); exact diagonal masks, 4 per slot
  - finalize without PE transposes: cast O^T to bf16, DMA-XBAR
    transpose to natural [q,h], reciprocal+scale on DVE, fp32 out
"""

import numpy as np
import ml_dtypes

import concourse.bacc as bacc
import concourse.mybir as mybir
import concourse.tile as tile
from concourse.bass_utils import run_bass_kernel_spmd

# Problem dims
B, T, C, HS = 4, 4096, 1024, 64
P = 128           # partitions
CH = 512          # query-chunk width
NCH = T // CH     # 8 chunks
NSLOT = NCH // 2  # 4 local query slots per core
CSUB = C // P     # 8 contraction subtiles
NKT = T // P      # 32 key tiles total
NEG = -1.0e9

BF16 = mybir.dt.bfloat16
NPBF16 = ml_dtypes.bfloat16


def _build_program(debug=False):
    nc = bacc.Bacc("TRN2")
    f32 = mybir.dt.float32
    EXP = mybir.ActivationFunctionType.Exp
    if debug:
        dbg_kt = nc.dram_tensor("dbg_kt", [HS, T], BF16, kind="ExternalOutput").ap()
        dbg_v = nc.dram_tensor("dbg_v", [P, NKT, HS + 1], BF16, kind="ExternalOutput").ap()
        dbg_qt = nc.dram_tensor("dbg_qt", [HS, NSLOT, CH], BF16, kind="ExternalOutput").ap()
        dbg_onat = nc.dram_tensor("dbg_onat", [NSLOT, P, 4, 80], BF16, kind="ExternalOutput").ap()

    xT = nc.dram_tensor("xT", [C, T], BF16, kind="ExternalInput").ap()
    xqT = nc.dram_tensor("xqT", [C, NSLOT * CH], BF16, kind="ExternalInput").ap()
    wkv = nc.dram_tensor("wkv", [C, 2 * HS], BF16, kind="ExternalInput").ap()
    wq = nc.dram_tensor("wq", [C, HS], BF16, kind="ExternalInput").ap()
    # additive causal masks for the last 8 key tiles of each slot's range,
    # as 4 pair-tiles of [128, 2, 512] (parity-dependent host data)
    mask_d = nc.dram_tensor("mask", [P, 8, CH], BF16, kind="ExternalInput").ap()
    out_d = nc.dram_tensor("out", [NSLOT * 4, P, HS], f32, kind="ExternalOutput").ap()

    # out viewed as [j][p, sub, h] to write one batched DMA per slot
    out_r = out_d.rearrange("(j s) p h -> j p s h", s=4)   # [4, 128, 4, 64]

    xT_r = xT.rearrange("(co ci) t -> ci co t", ci=P)      # [128, 8, 4096]
    xqT_r = xqT.rearrange("(co ci) t -> ci co t", ci=P)    # [128, 8, 2048]
    wkv_r = wkv.rearrange("(co ci) m -> ci co m", ci=P)    # [128, 8, 128]
    wq_r = wq.rearrange("(co ci) m -> ci co m", ci=P)      # [128, 8, 64]

    with tile.TileContext(nc) as tc:
        with (
            tc.tile_pool(name="const", bufs=1) as const_pool,
            tc.tile_pool(name="persist", bufs=1) as persist,
            tc.tile_pool(name="xin", bufs=6) as xpool,
            tc.tile_pool(name="xq", bufs=4) as xqpool,
            tc.tile_pool(name="vt", bufs=3) as vt_pool,
            tc.tile_pool(name="pt", bufs=4) as pt_pool,
            tc.tile_pool(name="osb", bufs=2) as osb_pool,
            tc.tile_pool(name="onat", bufs=2) as onat_pool,
            tc.tile_pool(name="fin", bufs=4) as fin_pool,
            tc.tile_pool(name="proj_ps", bufs=2, space="PSUM") as proj_ps,
            tc.tile_pool(name="st_ps", bufs=2, space="PSUM") as st_ps,
            tc.tile_pool(name="ot_ps", bufs=2, space="PSUM") as ot_ps,
        ):
            # ---- constants / persistent state ----
            # const DMAs go on the ACT hwdge queue so they never queue behind
            # the x-chunk stream on the sync queue (nor vice versa)
            wkv_sb = const_pool.tile([P, CSUB, 2 * HS], BF16)
            wq_sb = const_pool.tile([P, CSUB, HS], BF16)
            mask_sb = const_pool.tile([P, 8, CH], BF16)
            nc.scalar.dma_start(wkv_sb[:], wkv_r)
            nc.scalar.dma_start(wq_sb[:], wq_r)
            nc.scalar.dma_start(mask_sb[:], mask_d)

            kt_all = persist.tile([HS, T], BF16)             # K^T
            qt_slot = persist.tile([HS, NSLOT, CH], BF16)    # owned Q^T per slot
            v_all = persist.tile([P, NKT, HS + 1], BF16)     # V with ones column
            # 0x3F80 = bf16 1.0
            nc.vector.memset(
                v_all[:, :, HS : HS + 1].bitcast(mybir.dt.uint16), 0x3F80
            )

            for c in range(NCH):
                # ---- stacked K/V projection for every chunk ----
                xc = xpool.tile([P, CSUB, CH], BF16, tag="xc")
                nc.sync.dma_start(xc[:], xT_r[:, :, c * CH : (c + 1) * CH])

                kv_ps = proj_ps.tile([P, CH], f32, tag="proj")
                for cs in range(CSUB):
                    nc.tensor.matmul(
                        kv_ps[:],
                        lhsT=wkv_sb[:, cs, :],
                        rhs=xc[:, cs, :],
                        start=(cs == 0),
                        stop=(cs == CSUB - 1),
                    )
                nc.vector.tensor_copy(kt_all[:, c * CH : (c + 1) * CH], kv_ps[0:HS, :])
                vt = vt_pool.tile([HS, CH], BF16, tag="vt")
                nc.vector.tensor_copy(vt[:], kv_ps[HS:P, :])
                # V^T [64, 512] -> V natural [128, 4, 64] via DMA XBAR transpose.
                # XBAR ucode requires a contiguous output tile (it ignores
                # middle-dim strides), so stage then strided-copy into v_all.
                vst = vt_pool.tile([P, 4, HS], BF16, tag="vstage")
                nc.sync.dma_start_transpose(vst[:], vt[:])
                nc.gpsimd.tensor_copy(v_all[:, c * 4 : (c + 1) * 4, 0:HS], vst[:])

                if c % 2 == 0:
                    continue
                # ---- at odd chunks: slot j = (c-1)//2 ----
                j = (c - 1) // 2

                # Q^T projection from the owned-chunk x upload
                xq = xqpool.tile([P, CSUB, CH], BF16, tag="xq")
                nc.sync.dma_start(xq[:], xqT_r[:, :, j * CH : (j + 1) * CH])
                q_ps = proj_ps.tile([P, CH], f32, tag="proj")
                for cs in range(CSUB):
                    nc.tensor.matmul(
                        q_ps[0:HS, :],
                        lhsT=wq_sb[:, cs, :],
                        rhs=xq[:, cs, :],
                        start=(cs == 0),
                        stop=(cs == CSUB - 1),
                    )
                nc.vector.tensor_copy(qt_slot[:, j, :], q_ps[0:HS, :])

                # ---- attention for slot j over key-tile pairs ----
                nk = 8 * j + 8
                npair = nk // 2
                ot = ot_ps.tile([P, CH], f32, tag="ot")
                for pr in range(npair):
                    st = st_ps.tile([P, 2, CH], f32, tag="st")
                    for u in range(2):
                        kt = 2 * pr + u
                        nc.tensor.matmul(
                            st[:, u, :],
                            lhsT=kt_all[:, kt * P : (kt + 1) * P],
                            rhs=qt_slot[:, j, :],
                            start=True,
                            stop=True,
                        )
                    if pr >= npair - 4:  # masked band: last 8 kts of the range
                        d = pr - (npair - 4)
                        nc.vector.tensor_add(
                            st[:], st[:], mask_sb[:, 2 * d : 2 * d + 2, :]
                        )
                    pt = pt_pool.tile([P, 2, CH], BF16, tag="pt")
                    nc.scalar.activation(pt[:], st[:], EXP, scale=float(HS) ** -0.5)
                    for u in range(2):
                        kt = 2 * pr + u
                        nc.tensor.matmul(
                            ot[0 : HS + 1, :],
                            lhsT=v_all[:, kt, :],
                            rhs=pt[:, u, :],
                            start=(kt == 0),
                            stop=(kt == nk - 1),
                        )

                # ---- finalize slot j: cast, DMA-transpose, normalize ----
                o_sb = osb_pool.tile([80, CH], BF16, tag="osb")
                nc.vector.tensor_copy(o_sb[0 : HS + 1, :], ot[0 : HS + 1, :])
                onat = onat_pool.tile([P, 4, 80], BF16, tag="onat")
                nc.sync.dma_start_transpose(onat[:], o_sb[:])
                if debug:
                    nc.sync.dma_start(dbg_onat[j], onat[:])
                fo = fin_pool.tile([P, 4, HS], f32, tag="fo")
                for sub in range(4):
                    rsum = fin_pool.tile([P, 1], f32, tag="rsum")
                    nc.vector.tensor_copy(rsum[:], onat[:, sub, HS : HS + 1])
                    rec = fin_pool.tile([P, 1], f32, tag="rec")
                    nc.vector.reciprocal(rec[:], rsum[:])
                    nc.vector.tensor_scalar_mul(fo[:, sub, :], onat[:, sub, 0:HS], rec[:])
                nc.sync.dma_start(out_r[j], fo[:])

            if debug:
                nc.sync.dma_start(dbg_kt, kt_all[:])
                nc.sync.dma_start(dbg_v, v_all[:])
                nc.sync.dma_start(dbg_qt, qt_slot[:])

    nc.compile()
    return nc


_CACHE = {}


def _get_program():
    if "nc" not in _CACHE:
        _CACHE["nc"] = _build_program()
    return _CACHE["nc"]


def _host_inputs(x, Wk, Wq, Wv):
    x = np.asarray(x, dtype=np.float32)
    wkv = np.ascontiguousarray(
        np.concatenate([np.asarray(Wk), np.asarray(Wv)], axis=1)
    ).astype(NPBF16)
    wq = np.ascontiguousarray(np.asarray(Wq), dtype=np.float32).astype(NPBF16)

    xT = [np.ascontiguousarray(x[b].T).astype(NPBF16) for b in range(B)]
    # owned-chunk x, concatenated along t: core parity p owns chunks 2j+p
    xqT = {}
    for b in range(B):
        for p in range(2):
            cols = np.concatenate(
                [xT[b][:, (2 * j + p) * CH : (2 * j + p + 1) * CH] for j in range(NSLOT)],
                axis=1,
            )
            xqT[(b, p)] = np.ascontiguousarray(cols)

    # masks: for slot j (owned global chunk g = 2j+p), program kt range is
    # [0, 8j+8); the last 8 kts (4 pairs) get an additive mask tile.
    # pair-kt index within band: e = 0..7 maps to kt = 8j + e.
    # true causal: q (global col g*512+q) attends key (8j+e)*128+kk iff
    #   g*512 + q >= (8j+e)*128 + kk  <=>  q >= (e - 4p)*128 + kk - 512*? ...
    # with g = 2j+p: g*512 = 8j*128 + p*512, so condition is
    #   q + p*512 >= e*128 + kk  <=>  q >= (e*128 + kk - 512*p)
    masks = []
    ii = np.arange(P)
    qq = np.arange(CH)
    for p in range(2):
        m = np.empty((P, 8, CH), dtype=np.float32)
        for e in range(8):
            thr = e * 128 + ii[:, None] - 512 * p  # [128, 1]
            m[:, e, :] = np.where(qq[None, :] >= thr, 0.0, NEG)
        masks.append(m.astype(NPBF16))

    in_maps = []
    for core in range(2 * B):
        b, p = core // 2, core % 2
        in_maps.append(
            {
                "xT": xT[b],
                "xqT": xqT[(b, p)],
                "wkv": wkv,
                "wq": wq,
                "mask": masks[p],
            }
        )
    return in_maps


def _assemble(results):
    out = np.empty((B, T, HS), dtype=np.float32)
    for core in range(2 * B):
        b, p = core // 2, core % 2
        oc = results[core]["out"]  # [16, 128, 64]
        for j in range(NSLOT):
            g = 2 * j + p
            for sub in range(4):
                r0 = g * CH + sub * P
                out[b, r0 : r0 + P, :] = oc[j * 4 + sub]
    return out


def run(x, Wk, Wq, Wv, trace=False):
    nc = _get_program()
    in_maps = _host_inputs(x, Wk, Wq, Wv)
    res = run_bass_kernel_spmd(nc, in_maps, list(range(2 * B)), trace=trace)
    return _assemble(res.results), res


def kernel(x, Wk, Wq, Wv):
    out, _ = run(x, Wk, Wq, Wv)
    return out


# revision 22
# speedup vs baseline: 1.1148x; 1.1148x over previous
"""Causal single-head attention on 8 Trainium2 NeuronCores.

Problem: x[4, 4096, 1024], Wq/Wk/Wv[1024, 64] ->
  out = softmax(causal(Q K^T / 8)) V   per batch, fp32.

Sharding: core i handles batch b = i//2 with query-chunk parity p = i%2
(512-wide query chunks; core p owns global chunks {p, 2+p, 4+p, 6+p}).
SPMD: the program is identical on all cores; parity enters only through
data (which x chunks appear in xqT, the additive mask buffer, and host
assembly).

v2 design (from trace analysis of the fp32r baseline: fp32r streams 2
cycles/column on the PE, LDWEIGHTS churn dominated the V projection,
and ACT bubbles were paid per 512-col exp tile):
  - all matmul operands in bf16 (1 cy/col expected): halves x DMA too
  - K and V projected together as one stacked [Wk|Wv] matmul per chunk
    (V^T rows 64:128); V^T -> V natural via DMA-engine XBAR transpose
  - Q projected only for the 4 owned chunks from a separate xqT upload
    (uniform program; +4MiB DMA instead of +16.4k PE columns)
  - attention processes key tiles in PAIRS: one [128,1024] exp per pair
    (halves ACT instruction bubbles); exact diagonal masks, 4 per slot
  - finalize without PE transposes: cast O^T to bf16, DMA-XBAR
    transpose to natural [q,h], reciprocal+scale on DVE, fp32 out
"""

import numpy as np
import ml_dtypes

import concourse.bacc as bacc
import concourse.mybir as mybir
import concourse.tile as tile
from concourse.bass_utils import run_bass_kernel_spmd

# Problem dims
B, T, C, HS = 4, 4096, 1024, 64
P = 128           # partitions
CH = 512          # query-chunk width
NCH = T // CH     # 8 chunks
NSLOT = NCH // 2  # 4 local query slots per core
CSUB = C // P     # 8 contraction subtiles
NKT = T // P      # 32 key tiles total
NEG = -1.0e9

BF16 = mybir.dt.bfloat16
NPBF16 = ml_dtypes.bfloat16


def _build_program(debug=False):
    nc = bacc.Bacc("TRN2")
    f32 = mybir.dt.float32
    EXP = mybir.ActivationFunctionType.Exp
    if debug:
        dbg_kt = nc.dram_tensor("dbg_kt", [HS, T], BF16, kind="ExternalOutput").ap()
        dbg_v = nc.dram_tensor("dbg_v", [P, NKT, HS + 1], BF16, kind="ExternalOutput").ap()
        dbg_qt = nc.dram_tensor("dbg_qt", [HS, NSLOT, CH], BF16, kind="ExternalOutput").ap()
        dbg_onat = nc.dram_tensor("dbg_onat", [NSLOT, P, 4, 80], BF16, kind="ExternalOutput").ap()

    # x relaid on host as [chunk][partition][csub][t'] so each partition's
    # chunk slice is one contiguous 8KB run (1KB runs were packet-rate-bound
    # at ~106 GB/s on the DMA queue)
    xR = nc.dram_tensor("xR", [NCH, P, CSUB, CH], BF16, kind="ExternalInput").ap()
    xqR = nc.dram_tensor("xqR", [NSLOT, P, CSUB, CH], BF16, kind="ExternalInput").ap()
    wkv = nc.dram_tensor("wkv", [P, CSUB, 2 * HS], BF16, kind="ExternalInput").ap()
    wq = nc.dram_tensor("wq", [P, CSUB, HS], BF16, kind="ExternalInput").ap()
    # additive causal masks for the last 8 key tiles of each slot's range,
    # as 4 pair-tiles of [128, 2, 512] (parity-dependent host data)
    mask_d = nc.dram_tensor("mask", [P, 8, CH], BF16, kind="ExternalInput").ap()
    out_d = nc.dram_tensor("out", [NSLOT * 4, P, HS], f32, kind="ExternalOutput").ap()

    # out viewed as [j][p, sub, h] to write one batched DMA per slot
    out_r = out_d.rearrange("(j s) p h -> j p s h", s=4)   # [4, 128, 4, 64]

    with tile.TileContext(nc) as tc:
        with (
            tc.tile_pool(name="const", bufs=1) as const_pool,
            tc.tile_pool(name="persist", bufs=1) as persist,
            tc.tile_pool(name="xin", bufs=6) as xpool,
            tc.tile_pool(name="xq", bufs=4) as xqpool,
            tc.tile_pool(name="vt", bufs=3) as vt_pool,
            tc.tile_pool(name="pt", bufs=4) as pt_pool,
            tc.tile_pool(name="osb", bufs=2) as osb_pool,
            tc.tile_pool(name="onat", bufs=2) as onat_pool,
            tc.tile_pool(name="fin", bufs=4) as fin_pool,
            tc.tile_pool(name="proj_ps", bufs=2, space="PSUM") as proj_ps,
            tc.tile_pool(name="st_ps", bufs=2, space="PSUM") as st_ps,
            tc.tile_pool(name="ot_ps", bufs=2, space="PSUM") as ot_ps,
        ):
            # ---- constants / persistent state ----
            # const DMAs go on the ACT hwdge queue so they never queue behind
            # the x-chunk stream on the sync queue (nor vice versa)
            wkv_sb = const_pool.tile([P, CSUB, 2 * HS], BF16)
            wq_sb = const_pool.tile([P, CSUB, HS], BF16)
            mask_sb = const_pool.tile([P, 8, CH], BF16)
            nc.scalar.dma_start(wkv_sb[:], wkv)
            nc.scalar.dma_start(wq_sb[:], wq)
            nc.scalar.dma_start(mask_sb[:], mask_d)

            kt_all = persist.tile([HS, T], BF16)             # K^T
            qt_slot = persist.tile([HS, NSLOT, CH], BF16)    # owned Q^T per slot
            v_all = persist.tile([P, NKT, HS + 1], BF16)     # V with ones column
            # 0x3F80 = bf16 1.0
            nc.vector.memset(
                v_all[:, :, HS : HS + 1].bitcast(mybir.dt.uint16), 0x3F80
            )

            for c in range(NCH):
                # ---- stacked K/V projection for every chunk ----
                xc = xpool.tile([P, CSUB, CH], BF16, tag="xc")
                nc.sync.dma_start(xc[:], xR[c])

                kv_ps = proj_ps.tile([P, CH], f32, tag="proj")
                for cs in range(CSUB):
                    nc.tensor.matmul(
                        kv_ps[:],
                        lhsT=wkv_sb[:, cs, :],
                        rhs=xc[:, cs, :],
                        start=(cs == 0),
                        stop=(cs == CSUB - 1),
                    )
                nc.vector.tensor_copy(kt_all[:, c * CH : (c + 1) * CH], kv_ps[0:HS, :])
                vt = vt_pool.tile([HS, CH], BF16, tag="vt")
                nc.vector.tensor_copy(vt[:], kv_ps[HS:P, :])
                # V^T [64, 512] -> V natural [128, 4, 64] via DMA XBAR transpose.
                # XBAR ucode requires a contiguous output tile (it ignores
                # middle-dim strides), so stage then strided-copy into v_all.
                vst = vt_pool.tile([P, 4, HS], BF16, tag="vstage")
                nc.sync.dma_start_transpose(vst[:], vt[:])
                nc.gpsimd.tensor_copy(v_all[:, c * 4 : (c + 1) * 4, 0:HS], vst[:])

                if c % 2 == 0:
                    continue
                # ---- at odd chunks: slot j = (c-1)//2 ----
                j = (c - 1) // 2

                # Q^T projection from the owned-chunk x upload
                xq = xqpool.tile([P, CSUB, CH], BF16, tag="xq")
                nc.sync.dma_start(xq[:], xqR[j])
                q_ps = proj_ps.tile([P, CH], f32, tag="proj")
                for cs in range(CSUB):
                    nc.tensor.matmul(
                        q_ps[0:HS, :],
                        lhsT=wq_sb[:, cs, :],
                        rhs=xq[:, cs, :],
                        start=(cs == 0),
                        stop=(cs == CSUB - 1),
                    )
                nc.vector.tensor_copy(qt_slot[:, j, :], q_ps[0:HS, :])

                # ---- attention for slot j over key-tile pairs ----
                nk = 8 * j + 8
                npair = nk // 2
                ot = ot_ps.tile([P, CH], f32, tag="ot")
                for pr in range(npair):
                    st = st_ps.tile([P, 2, CH], f32, tag="st")
                    for u in range(2):
                        kt = 2 * pr + u
                        nc.tensor.matmul(
                            st[:, u, :],
                            lhsT=kt_all[:, kt * P : (kt + 1) * P],
                            rhs=qt_slot[:, j, :],
                            start=True,
                            stop=True,
                        )
                    if pr >= npair - 4:  # masked band: last 8 kts of the range
                        d = pr - (npair - 4)
                        nc.vector.tensor_add(
                            st[:], st[:], mask_sb[:, 2 * d : 2 * d + 2, :]
                        )
                    pt = pt_pool.tile([P, 2, CH], BF16, tag="pt")
                    nc.scalar.activation(pt[:], st[:], EXP, scale=float(HS) ** -0.5)
                    for u in range(2):
                        kt = 2 * pr + u
                        nc.tensor.matmul(
                            ot[0 : HS + 1, :],
                            lhsT=v_all[:, kt, :],
                            rhs=pt[:, u, :],
                            start=(kt == 0),
                            stop=(kt == nk - 1),
                        )

                # ---- finalize slot j: cast, DMA-transpose, normalize ----
                o_sb = osb_pool.tile([80, CH], BF16, tag="osb")
                nc.vector.tensor_copy(o_sb[0 : HS + 1, :], ot[0 : HS + 1, :])
                onat = onat_pool.tile([P, 4, 80], BF16, tag="onat")
                nc.sync.dma_start_transpose(onat[:], o_sb[:])
                if debug:
                    nc.sync.dma_start(dbg_onat[j], onat[:])
                rsum = fin_pool.tile([P, 4], f32, tag="rsum")
                nc.vector.tensor_copy(rsum[:], onat[:, :, HS])
                rec = fin_pool.tile([P, 4], f32, tag="rec")
                nc.vector.reciprocal(rec[:], rsum[:])
                fo = fin_pool.tile([P, 4, HS], f32, tag="fo")
                for sub in range(4):
                    nc.vector.tensor_scalar_mul(
                        fo[:, sub, :], onat[:, sub, 0:HS], rec[:, sub : sub + 1]
                    )
                nc.sync.dma_start(out_r[j], fo[:])

            if debug:
                nc.sync.dma_start(dbg_kt, kt_all[:])
                nc.sync.dma_start(dbg_v, v_all[:])
                nc.sync.dma_start(dbg_qt, qt_slot[:])

    nc.compile()
    return nc


_CACHE = {}


def _get_program():
    if "nc" not in _CACHE:
        _CACHE["nc"] = _build_program()
    return _CACHE["nc"]


def _host_inputs(x, Wk, Wq, Wv):
    x = np.asarray(x, dtype=np.float32)
    # weights relaid to [partition, csub, m] so SBUF rows are contiguous runs
    wkv = np.concatenate([np.asarray(Wk), np.asarray(Wv)], axis=1)  # [1024, 128]
    wkv = np.ascontiguousarray(
        wkv.reshape(CSUB, P, 2 * HS).transpose(1, 0, 2)
    ).astype(NPBF16)
    wq = np.ascontiguousarray(
        np.asarray(Wq).reshape(CSUB, P, HS).transpose(1, 0, 2)
    ).astype(NPBF16)

    # x^T relaid to [chunk, partition, csub, t'] (contiguous 8KB per partition)
    xT = [np.ascontiguousarray(x[b].T).astype(NPBF16) for b in range(B)]
    xR = {}
    xqR = {}
    for b in range(B):
        # xT[b] is [C, T] = [(csub, ci), (c, t')]
        r = xT[b].reshape(CSUB, P, NCH, CH).transpose(2, 1, 0, 3)
        xR[b] = np.ascontiguousarray(r)  # [8, 128, 8, 512]
        for p in range(2):
            xqR[(b, p)] = np.ascontiguousarray(
                r[[2 * j + p for j in range(NSLOT)]]
            )  # [4, 128, 8, 512]

    # masks: for slot j (owned global chunk g = 2j+p), program kt range is
    # [0, 8j+8); the last 8 kts (4 pairs) get an additive mask tile.
    # pair-kt index within band: e = 0..7 maps to kt = 8j + e.
    # true causal: q (global col g*512+q) attends key (8j+e)*128+kk iff
    #   g*512 + q >= (8j+e)*128 + kk  <=>  q >= (e - 4p)*128 + kk - 512*? ...
    # with g = 2j+p: g*512 = 8j*128 + p*512, so condition is
    #   q + p*512 >= e*128 + kk  <=>  q >= (e*128 + kk - 512*p)
    masks = []
    ii = np.arange(P)
    qq = np.arange(CH)
    for p in range(2):
        m = np.empty((P, 8, CH), dtype=np.float32)
        for e in range(8):
            thr = e * 128 + ii[:, None] - 512 * p  # [128, 1]
            m[:, e, :] = np.where(qq[None, :] >= thr, 0.0, NEG)
        masks.append(m.astype(NPBF16))

    in_maps = []
    for core in range(2 * B):
        b, p = core // 2, core % 2
        in_maps.append(
            {
                "xR": xR[b],
                "xqR": xqR[(b, p)],
                "wkv": wkv,
                "wq": wq,
                "mask": masks[p],
            }
        )
    return in_maps


def _assemble(results):
    out = np.empty((B, T, HS), dtype=np.float32)
    for core in range(2 * B):
        b, p = core // 2, core % 2
        oc = results[core]["out"]  # [16, 128, 64]
        for j in range(NSLOT):
            g = 2 * j + p
            for sub in range(4):
                r0 = g * CH + sub * P
                out[b, r0 : r0 + P, :] = oc[j * 4 + sub]
    return out


def run(x, Wk, Wq, Wv, trace=False):
    nc = _get_program()
    in_maps = _host_inputs(x, Wk, Wq, Wv)
    res = run_bass_kernel_spmd(nc, in_maps, list(range(2 * B)), trace=trace)
    return _assemble(res.results), res


def kernel(x, Wk, Wq, Wv):
    out, _ = run(x, Wk, Wq, Wv)
    return out
